# revision 13
# baseline (speedup 1.0000x reference)
"""3-layer GAT on 8 Trainium2 NeuronCores — v2 (single-gather edge phase).

Sharding: dst-block edge sharding. Core c owns dst nodes [c*6250,(c+1)*6250)
(padded to 6272 = 49 windows x 128) and all edges pointing into them.

Per layer l the node table is bf16 [h_l (128) | alsrc_l (4)] rows, built
distributed (each core computes its own 49-window block) and AllGathered in
two segments (A = windows 0:25 of every core, B = windows 25:49) so int16
gather indices cover each segment (25600 / 24576 rows).

Edge phase (per 16-chunk batch of 2048 edges):
  - ONE dma_gather of [h|alsrc] rows by src (512B rows) — the only per-edge
    SWDGE work.  aldst is NOT gathered: per chunk, a one-hot S_T matrix
    ([dst_rel partition, edge free], built by is_equal against the replicated
    dst_rel stream) broadcasts the window's aldst values to edges via a
    4-column matmul on the otherwise idle tensor engine.
  - score = exp(prelu(alsrc + aldst)); Exp writes wx straight into the
    gathered tile's alsrc columns (scalar engine), messages are scaled
    in place per head, and one matmul per chunk accumulates
    [wx*h | wx] into the window's PSUM acc via the one-hot S matrix.
Flush divides by the accumulated denominator, adds bias, relu, and
immediately computes the next layer's [h|alsrc|aldst] block row
(feeding the next segment AllGathers, emitted when windows 24/48 flush).
Host-side work is integer-only: edge grouping, padding, index packing.
"""
import numpy as np

N_CORES = 8
NB_REAL = 6250
NW = 49
NBP = NW * 128            # 6272
WA, WB = 25, 24           # windows per table segment
RA, RB = WA * 128, WB * 128   # rows per core per segment: 3200 / 3072
GA, GB = N_CORES * RA, N_CORES * RB  # global segment rows: 25600 / 24576
BATCH_CH = 16             # chunks per gather batch (2048 edges)

LAST_EXEC_NS = None


# ----------------------------------------------------------------------------
# host-side integer preprocessing
# ----------------------------------------------------------------------------
def _build_schedule(edge_index):
    src = edge_index[0].astype(np.int64)
    dst = edge_index[1].astype(np.int64)
    core = dst // NB_REAL
    r = dst - core * NB_REAL
    w = r >> 7
    src_core = src // NB_REAL
    src_r = src - src_core * NB_REAL
    half = (src_r >= RA).astype(np.int64)
    src16 = np.where(half == 1, src_core * RB + (src_r - RA), src_core * RA + src_r)

    grp = (core * NW + w) * 2 + half
    counts = np.bincount(grp, minlength=N_CORES * NW * 2).reshape(N_CORES, NW, 2)
    n_ch = -(-counts.max(axis=0) // 128)          # [NW, 2]
    empty = n_ch.sum(axis=1) == 0
    n_ch[empty, 0] = 1

    ch_off = np.zeros((NW, 2), np.int64)
    ch_off[:, 0] = np.cumsum(n_ch[:, 0]) - n_ch[:, 0]
    ch_off[:, 1] = np.cumsum(n_ch[:, 1]) - n_ch[:, 1]
    nch_stream = [int(n_ch[:, 0].sum()), int(n_ch[:, 1].sum())]
    win_of = [np.repeat(np.arange(NW), n_ch[:, 0]),
              np.repeat(np.arange(NW), n_ch[:, 1])]

    schedule = []
    for wi in range(NW):
        nwch = int(n_ch[wi, 0] + n_ch[wi, 1])
        k = 0
        for h in range(2):
            for j in range(int(n_ch[wi, h])):
                schedule.append((wi, h, int(ch_off[wi, h] + j), k == 0, k == nwch - 1))
                k += 1

    per_core = []
    for c in range(N_CORES):
        m = core == c
        sc16, dloc, hh, ww = src16[m], r[m], half[m], w[m]
        arrs = {}
        for h in range(2):
            nslots = nch_stream[h] * 128
            a_idx = np.zeros(nslots, np.int16)
            a_rel = np.full(nslots, -1, np.int8)
            hm = hh == h
            e_s, e_d, e_w = sc16[hm], dloc[hm], ww[hm]
            order = np.argsort(e_w, kind="stable")
            e_s, e_d, e_w = e_s[order], e_d[order], e_w[order]
            cnts = np.bincount(e_w, minlength=NW)
            starts = np.cumsum(cnts) - cnts
            rank = np.arange(len(e_w)) - starts[e_w]
            slot = ch_off[e_w, h] * 128 + rank
            a_idx[slot] = e_s.astype(np.int16)
            a_rel[slot] = (e_d & 127).astype(np.int8)
            arrs[h] = (a_idx, a_rel)
        per_core.append(arrs)

    return {"n_ch": n_ch, "ch_off": ch_off, "nch_stream": nch_stream,
            "schedule": schedule, "win_of": win_of, "per_core": per_core}


def _pack_idx16(arr):
    assert len(arr) % 16 == 0
    return np.ascontiguousarray(np.tile(arr.reshape(-1, 16).T, (8, 1)))


# ----------------------------------------------------------------------------
# bass program
# ----------------------------------------------------------------------------
def _build_program(sch):
    import os
    SINGLE_PACKET = os.environ.get("GAT_SP", "0") == "1"
    PREF_A = int(os.environ.get("GAT_PREF", "5"))
    NQ = int(os.environ.get("GAT_NQ", "1"))
    import concourse.bacc as bacc
    import concourse.mybir as mybir
    from concourse import tile

    f32 = mybir.dt.float32
    bf16 = mybir.dt.bfloat16
    i16 = mybir.dt.int16
    i8 = mybir.dt.int8
    nchA, nchB = sch["nch_stream"]
    tot_ch = nchA + nchB
    NSLOT = tot_ch * 128

    nc = bacc.Bacc("TRN2", target_bir_lowering=False, debug=False,
                   num_devices=N_CORES, num_swdge_queues=NQ)

    # external I/O
    xbT = nc.dram_tensor("x_blkT", (128, NBP), f32, kind="ExternalInput")
    iA_d = nc.dram_tensor("idxA", (128, nchA * 8), i16, kind="ExternalInput")
    iB_d = nc.dram_tensor("idxB", (128, nchB * 8), i16, kind="ExternalInput")
    rel_d = nc.dram_tensor("rel2d", (128, tot_ch), i8, kind="ExternalInput")
    relT_d = nc.dram_tensor("relT", (128, NSLOT), i8, kind="ExternalInput")
    iotar_d = nc.dram_tensor("iotar", (128, 128), i8, kind="ExternalInput")
    iotac_d = nc.dram_tensor("iotac", (128, 1), i8, kind="ExternalInput")
    eye_d = nc.dram_tensor("eye", (128, 128), f32, kind="ExternalInput")
    W1_d = nc.dram_tensor("W1", (128, 128), f32, kind="ExternalInput")
    W2_d = nc.dram_tensor("W2", (128, 128), f32, kind="ExternalInput")
    W3_d = nc.dram_tensor("W3", (128, 2), f32, kind="ExternalInput")
    ac1_d = nc.dram_tensor("acat1", (128, 8), f32, kind="ExternalInput")
    ac2_d = nc.dram_tensor("acat2", (128, 8), f32, kind="ExternalInput")
    ac3_d = nc.dram_tensor("acat3", (2, 2), f32, kind="ExternalInput")
    b1_d = nc.dram_tensor("bias1", (128, 128), f32, kind="ExternalInput")
    b2_d = nc.dram_tensor("bias2", (128, 128), f32, kind="ExternalInput")
    b3_d = nc.dram_tensor("bias3", (128, 2), f32, kind="ExternalInput")
    out3_d = nc.dram_tensor("out3", (NBP, 2), f32, kind="ExternalOutput")

    AluOp = mybir.AluOpType
    Act = mybir.ActivationFunctionType
    RG = [list(range(N_CORES))]

    with tile.TileContext(nc) as tc:
        with (
            tc.tile_pool(name="const", bufs=1) as pc,
            tc.tile_pool(name="idxp", bufs=1) as pidx,
            tc.tile_pool(name="adp", bufs=1) as pads,
            tc.tile_pool(name="batch", bufs=3) as pb,
            tc.tile_pool(name="p1", bufs=3) as p1,
            tc.tile_pool(name="flush", bufs=2) as pf,
            tc.tile_pool(name="pw", bufs=2, space="PSUM") as pw,
            tc.tile_pool(name="pe", bufs=2, space="PSUM") as pe,
            tc.tile_pool(name="pt", bufs=2, space="PSUM") as pt,
            tc.tile_pool(name="ph", bufs=2, space="PSUM") as ph,
            tc.tile_pool(name="dram", bufs=1, space="DRAM") as pd,
        ):
            # persistent DRAM tables: local blocks + gathered segments
            l1A = pd.tile([RA, 256], bf16, name="l1A")
            l1B = pd.tile([RB, 256], bf16, name="l1B")
            t1A = pd.tile([GA, 256], bf16, name="t1A", addr_space="Shared")
            t1B = pd.tile([GB, 256], bf16, name="t1B", addr_space="Shared")
            l2A = pd.tile([RA, 256], bf16, name="l2A")
            l2B = pd.tile([RB, 256], bf16, name="l2B")
            t2A = pd.tile([GA, 256], bf16, name="t2A", addr_space="Shared")
            t2B = pd.tile([GB, 256], bf16, name="t2B", addr_space="Shared")
            l3A = pd.tile([RA, 128], bf16, name="l3A")
            l3B = pd.tile([RB, 128], bf16, name="l3B")
            t3A = pd.tile([GA, 128], bf16, name="t3A", addr_space="Shared")
            t3B = pd.tile([GB, 128], bf16, name="t3B", addr_space="Shared")

            # constants to SBUF
            def load_const(name, dram, shape, dt=f32):
                t = pc.tile(shape, dt, name=name)
                nc.sync.dma_start(out=t[:], in_=dram[:])
                return t

            iotar = load_const("iotar_sb", iotar_d, [128, 128], i8)
            iotac = load_const("iotac_sb", iotac_d, [128, 1], i8)
            eye = load_const("eye_sb", eye_d, [128, 128])
            W1 = load_const("W1_sb", W1_d, [128, 128])
            W2 = load_const("W2_sb", W2_d, [128, 128])
            W3 = load_const("W3_sb", W3_d, [128, 2])
            ac1 = load_const("ac1_sb", ac1_d, [128, 8])
            ac2 = load_const("ac2_sb", ac2_d, [128, 8])
            ac3 = load_const("ac3_sb", ac3_d, [2, 2])
            bias1 = load_const("bias1_sb", b1_d, [128, 128])
            bias2 = load_const("bias2_sb", b2_d, [128, 128])
            bias3 = load_const("bias3_sb", b3_d, [128, 2])

            iA = pidx.tile([128, nchA * 8], i16, name="iA")
            nc.sync.dma_start(out=iA[:], in_=iA_d[:])
            iB = pidx.tile([128, nchB * 8], i16, name="iB")
            nc.sync.dma_start(out=iB[:], in_=iB_d[:])
            rel = pidx.tile([128, tot_ch], i8, name="rel")
            nc.sync.dma_start(out=rel[:], in_=rel_d[:])

            # per-layer aldst tiles (window-local, never leave SBUF)
            ad1 = pads.tile([128, NW, 4], bf16, name="ad1")
            ad2 = pads.tile([128, NW, 4], bf16, name="ad2")
            ad3 = pads.tile([128, NW, 1], bf16, name="ad3")

            # setup: rhs_cat_l = [W_l | W_l @ acat_l]
            def make_rhs_cat(W, ac, name):
                tp = pt.tile([128, 128], f32, name=f"{name}_tp", tag="tpose")
                nc.tensor.transpose(tp[:], W[:], eye[:])
                WT = pc.tile([128, 128], f32, name=f"{name}_WT")
                nc.vector.tensor_copy(out=WT[:], in_=tp[:])
                rc = pc.tile([128, 136], f32, name=f"{name}_rc")
                nc.vector.tensor_copy(out=rc[:, 0:128], in_=W[:])
                wa = ph.tile([128, 8], f32, name=f"{name}_wa", tag="halp")
                nc.tensor.matmul(wa[:], WT[:], ac[:])
                nc.vector.tensor_copy(out=rc[:, 128:136], in_=wa[:])
                return rc

            rc1 = make_rhs_cat(W1, ac1, "rc1")
            rc2 = make_rhs_cat(W2, ac2, "rc2")
            # layer 3: rc3 = [W3 | W3 @ acat3]  -> [128, 4]
            tp3 = pt.tile([2, 128], f32, name="tp3", tag="tpose")
            nc.tensor.transpose(tp3[:], W3[:], eye[:])
            W3T = pc.tile([2, 128], f32, name="W3T")
            nc.vector.tensor_copy(out=W3T[:], in_=tp3[:])
            rc3 = pc.tile([128, 4], f32, name="rc3")
            nc.vector.tensor_copy(out=rc3[:, 0:2], in_=W3[:])
            wa3 = ph.tile([128, 2], f32, name="wa3", tag="halp")
            nc.tensor.matmul(wa3[:], W3T[:], ac3[:])
            nc.vector.tensor_copy(out=rc3[:, 2:4], in_=wa3[:])

            def emit_ag(locA, locB, tA, tB, which):
                if which == 0:
                    nc.gpsimd.collective_compute(
                        "AllGather", AluOp.bypass, replica_groups=RG,
                        ins=[locA.opt()], outs=[tA.opt()])
                else:
                    nc.gpsimd.collective_compute(
                        "AllGather", AluOp.bypass, replica_groups=RG,
                        ins=[locB.opt()], outs=[tB.opt()])

            # write one [h|as] block row + ad column from a [128, 132+4k] psum
            def store_block(hp, wi, hcols, nh, locA, locB, ad_next, tagsuf):
                halb = pf.tile([128, hcols + nh], bf16,
                               name=f"halb{tagsuf}", tag=f"halb{tagsuf}")
                nc.scalar.activation(out=halb[:, 0:hcols], in_=hp[:, 0:hcols],
                                     func=Act.Copy)
                nc.scalar.activation(out=halb[:, hcols:hcols + nh],
                                     in_=hp[:, hcols:hcols + nh], func=Act.Copy)
                nc.scalar.activation(out=ad_next[:, wi, :],
                                     in_=hp[:, hcols + nh:hcols + 2 * nh],
                                     func=Act.Copy)
                if wi < WA:
                    r0 = wi * 128
                    nc.sync.dma_start(out=locA[r0:r0 + 128, 0:hcols + nh],
                                      in_=halb[:])
                else:
                    r0 = (wi - WA) * 128
                    nc.sync.dma_start(out=locB[r0:r0 + 128, 0:hcols + nh],
                                      in_=halb[:])

            # ---------------- phase 1 (layer 1 projection, local block) ----
            for w in range(NW):
                xT = p1.tile([128, 128], f32, name="p1xt", tag="p1xt")
                nc.sync.dma_start(out=xT[:], in_=xbT[:, w * 128:(w + 1) * 128])
                hp = ph.tile([128, 136], f32, name="p1hp", tag="halp")
                nc.tensor.matmul(hp[:], xT[:], rc1[:])
                store_block(hp, w, 128, 4, l1A, l1B, ad1, "12")
                if w == WA - 1:
                    emit_ag(l1A, l1B, t1A, t1B, 0)
                if w == NW - 1:
                    emit_ag(l1A, l1B, t1A, t1B, 1)

            # ---------------- edge phase ----------------
            qctr = [0]

            def edge_layer(tA, tB, ad_cur, layer3, flush_fn):
                width = 128 if layer3 else 256
                pay = 3 if layer3 else 132
                nh = 1 if layer3 else 4
                hcol = 2 if layer3 else 128
                idx_s = {0: iA, 1: iB}
                tbl = {0: tA, 1: tB}
                doff = {0: 0, 1: nchA}
                win_of = sch["win_of"]
                batches = {}

                def materialize(h, b):
                    if (h, b) in batches:
                        return batches[(h, b)]
                    nch_s = nchA if h == 0 else nchB
                    c0, c1 = b * BATCH_CH, min((b + 1) * BATCH_CH, nch_s)
                    nb = c1 - c0
                    ni = nb * 128
                    s0 = (doff[h] + c0) * 128
                    g = pb.tile([128, BATCH_CH, width], bf16, name="g", tag="g",
                                bufs=8)
                    nc.gpsimd.dma_gather(
                        out_ap=g[:, 0:nb, :], in_ap=tbl[h][:, :],
                        idxs_ap=idx_s[h][:, c0 * 8:c1 * 8],
                        num_idxs=ni, num_idxs_reg=ni, elem_size=width,
                        single_packet=SINGLE_PACKET, queue_num=qctr[0] % NQ)
                    qctr[0] += 1
                    rT = pb.tile([128, BATCH_CH * 128], i8, name="rT", tag="rT")
                    nc.sync.dma_start(out=rT[:, 0:ni], in_=relT_d[:, s0:s0 + ni])
                    ST = pb.tile([128, BATCH_CH, 128], bf16, name="ST", tag="ST")
                    nc.vector.tensor_tensor(
                        out=ST[:, 0:nb, :],
                        in0=iotac[:][:, :, None].broadcast_to((128, nb, 128)),
                        in1=rT[:, 0:ni].rearrange("p (c e) -> p c e", e=128),
                        op=AluOp.is_equal)
                    eps = pe.tile([128, BATCH_CH, nh], f32, name="eps", tag="eps")
                    for c in range(nb):
                        wc = int(win_of[h][c0 + c])
                        nc.tensor.matmul(eps[:, c, :], ST[:, c, :],
                                         ad_cur[:, wc, :], start=True, stop=True)
                    sc0 = pb.tile([128, BATCH_CH, nh], bf16, name="sc0", tag="sc0")
                    nc.scalar.activation(out=sc0[:, 0:nb, :], in_=eps[:, 0:nb, :],
                                         func=Act.Copy)
                    sc = pb.tile([128, BATCH_CH, nh], bf16, name="sc", tag="sc")
                    nc.vector.tensor_tensor(
                        out=sc[:, 0:nb, :], in0=sc0[:, 0:nb, :],
                        in1=g[:, 0:nb, hcol:hcol + nh], op=AluOp.add)
                    scp = pb.tile([128, BATCH_CH, nh], bf16, name="scp", tag="scp")
                    nc.scalar.activation(out=scp[:, 0:nb, :], in_=sc[:, 0:nb, :],
                                         func=Act.Prelu, alpha=0.2)
                    nc.scalar.activation(out=g[:, 0:nb, hcol:hcol + nh],
                                         in_=scp[:, 0:nb, :], func=Act.Exp)
                    if not layer3:
                        nc.vector.tensor_tensor(
                            out=g[:, 0:nb, 0:128].rearrange(
                                "p n (h d) -> p n h d", d=32),
                            in0=g[:, 0:nb, 0:128].rearrange(
                                "p n (h d) -> p n h d", d=32),
                            in1=g[:, 0:nb, 128:132].rearrange(
                                "p n (h d) -> p n h d", d=1).broadcast_to(
                                    (128, nb, 4, 32)),
                            op=AluOp.mult)
                    else:
                        nc.vector.tensor_tensor(
                            out=g[:, 0:nb, 0:2], in0=g[:, 0:nb, 0:2],
                            in1=g[:, 0:nb, 2:3].broadcast_to((128, nb, 2)),
                            op=AluOp.mult)
                    S = pb.tile([128, BATCH_CH, 128], bf16, name="S", tag="S",
                                bufs=8)
                    nc.vector.tensor_tensor(
                        out=S[:, 0:nb, :],
                        in0=iotar[:][:, None, :].broadcast_to((128, nb, 128)),
                        in1=rel[:, doff[h] + c0:doff[h] + c1].broadcast_to(
                            (128, nb, 128)),
                        op=AluOp.is_equal)
                    batches[(h, b)] = (S, g)
                    return S, g

                # keep the A-stream gathers PREF batches ahead of consumption:
                # at layer entry only segment A's AllGather has landed, so the
                # run-ahead A gathers keep GpSimd busy while segment B flies.
                nbatchA = -(-nchA // BATCH_CH)
                state = {"nextA": 0}

                def prefetch_a(upto):
                    while state["nextA"] < min(nbatchA, upto):
                        materialize(0, state["nextA"])
                        state["nextA"] += 1

                prefetch_a(PREF_A)
                acc = None
                for (wi, h, pos, first, last) in sch["schedule"]:
                    b, col = pos // BATCH_CH, pos % BATCH_CH
                    if h == 1 and (h, b) not in batches:
                        # advance the A-stream run-ahead BEFORE emitting a
                        # B gather: B gathers block on segment-B's AllGather
                        # and would head-of-line-block A on the in-order
                        # GpSimd queue.
                        prefetch_a(state["nextA"] + 1)
                    S, g = materialize(h, b)
                    if h == 0 and b >= state["nextA"]:
                        state["nextA"] = b + 1
                    if first:
                        acc = pw.tile([128, pay], f32, name="acc", tag="acc")
                    nc.tensor.matmul(acc[:], S[:, col, :], g[:, col, 0:pay],
                                     start=first, stop=last)
                    if last:
                        flush_fn(wi, acc)

            # ---------------- flushes ----------------
            def make_flush12(rc_next, bias_t, locA, locB, tA, tB, ad_next,
                             next_hcols, next_nh):
                def flush(wi, acc):
                    den = pf.tile([128, 4], f32, name="den", tag="den")
                    nc.vector.tensor_scalar_max(out=den[:], in0=acc[:, 128:132],
                                                scalar1=1e-30)
                    rcp = pf.tile([128, 4], f32, name="rcp", tag="rcp")
                    nc.vector.reciprocal(out=rcp[:], in_=den[:])
                    outn = pf.tile([128, 128], f32, name="outn", tag="outn")
                    for hh in range(4):
                        nc.vector.scalar_tensor_tensor(
                            out=outn[:, hh * 32:(hh + 1) * 32],
                            in0=acc[:, hh * 32:(hh + 1) * 32],
                            scalar=rcp[:, hh:hh + 1],
                            in1=bias_t[:, hh * 32:(hh + 1) * 32],
                            op0=AluOp.mult, op1=AluOp.add)
                    rl = pf.tile([128, 128], f32, name="rl", tag="rl")
                    nc.scalar.activation(out=rl[:], in_=outn[:], func=Act.Relu)
                    tp = pt.tile([128, 128], f32, name="ftp", tag="tpose")
                    nc.tensor.transpose(tp[:], rl[:], eye[:])
                    rlT = pf.tile([128, 128], f32, name="rlT", tag="rlT")
                    nc.vector.tensor_copy(out=rlT[:], in_=tp[:])
                    ncols = next_hcols + 2 * next_nh
                    hp = ph.tile([128, ncols], f32, name="fhp", tag="halp")
                    nc.tensor.matmul(hp[:], rlT[:], rc_next[:, 0:ncols])
                    store_block(hp, wi, next_hcols, next_nh, locA, locB,
                                ad_next, "23" if next_hcols == 2 else "12")
                    if wi == WA - 1:
                        emit_ag(locA, locB, tA, tB, 0)
                    if wi == NW - 1:
                        emit_ag(locA, locB, tA, tB, 1)
                return flush

            def flush3(wi, acc):
                den = pf.tile([128, 1], f32, name="den3", tag="den3")
                nc.vector.tensor_scalar_max(out=den[:], in0=acc[:, 2:3],
                                            scalar1=1e-30)
                rcp = pf.tile([128, 1], f32, name="rcp3", tag="rcp3")
                nc.vector.reciprocal(out=rcp[:], in_=den[:])
                outn = pf.tile([128, 2], f32, name="outn3", tag="outn3")
                nc.vector.scalar_tensor_tensor(
                    out=outn[:], in0=acc[:, 0:2], scalar=rcp[:, 0:1],
                    in1=bias3[:, 0:2], op0=AluOp.mult, op1=AluOp.add)
                nc.sync.dma_start(out=out3_d[wi * 128:(wi + 1) * 128, :],
                                  in_=outn[:])

            # ---------------- run the three layers ----------------
            edge_layer(t1A, t1B, ad1, False,
                       make_flush12(rc2, bias1, l2A, l2B, t2A, t2B, ad2,
                                    128, 4))
            edge_layer(t2A, t2B, ad2, False,
                       make_flush12(rc3, bias2, l3A, l3B, t3A, t3B, ad3,
                                    2, 1))
            edge_layer(t3A, t3B, ad3, True, flush3)

    nc.compile()
    return nc


# ----------------------------------------------------------------------------
# entry point
# ----------------------------------------------------------------------------
def kernel(x, edge_index, W1, a_src1, a_dst1, b1, W2, a_src2, a_dst2, b2,
           W3, a_src3, a_dst3, b3, _trace=False):
    global LAST_EXEC_NS
    from concourse.bass_utils import run_bass_kernel_spmd

    x = np.asarray(x, np.float32)
    edge_index = np.asarray(edge_index)
    sch = _build_schedule(edge_index)
    nc = _build_program(sch)

    def acat_flat(a_src, a_dst, hid, heads, D):
        ac = np.zeros((hid, 2 * heads), np.float32)
        for h in range(heads):
            ac[h * D:(h + 1) * D, h] = a_src[h]
            ac[h * D:(h + 1) * D, heads + h] = a_dst[h]
        return ac

    ac1 = acat_flat(np.asarray(a_src1), np.asarray(a_dst1), 128, 4, 32)
    ac2 = acat_flat(np.asarray(a_src2), np.asarray(a_dst2), 128, 4, 32)
    ac3 = np.stack([np.asarray(a_src3)[0], np.asarray(a_dst3)[0]],
                   axis=1).astype(np.float32)

    base = {
        "iotar": np.tile(np.arange(128, dtype=np.int8), (128, 1)),
        "iotac": np.arange(128, dtype=np.int8).reshape(128, 1),
        "eye": np.eye(128, dtype=np.float32),
        "W1": np.asarray(W1, np.float32), "W2": np.asarray(W2, np.float32),
        "W3": np.asarray(W3, np.float32),
        "acat1": ac1, "acat2": ac2, "acat3": ac3,
        "bias1": np.tile(np.asarray(b1, np.float32), (128, 1)),
        "bias2": np.tile(np.asarray(b2, np.float32), (128, 1)),
        "bias3": np.tile(np.asarray(b3, np.float32), (128, 1)),
    }
    in_maps = []
    for c in range(N_CORES):
        a_idx, a_rel = sch["per_core"][c][0]
        b_idx, b_rel = sch["per_core"][c][1]
        rel_flat = np.concatenate([a_rel, b_rel])
        m = dict(base)
        xblk = np.zeros((NBP, 128), np.float32)
        xblk[0:NB_REAL] = x[c * NB_REAL:(c + 1) * NB_REAL]
        m["x_blkT"] = np.ascontiguousarray(xblk.T)
        m["idxA"] = _pack_idx16(a_idx)
        m["idxB"] = _pack_idx16(b_idx)
        m["rel2d"] = np.ascontiguousarray(rel_flat.reshape(-1, 128).T)
        m["relT"] = np.ascontiguousarray(np.tile(rel_flat, (128, 1)))
        in_maps.append(m)

    res = run_bass_kernel_spmd(nc, in_maps, list(range(N_CORES)), trace=_trace)
    LAST_EXEC_NS = res.exec_time_ns

    out = np.empty((N_CORES * NB_REAL, 2), np.float32)
    for c in range(N_CORES):
        out[c * NB_REAL:(c + 1) * NB_REAL] = \
            np.asarray(res.results[c]["out3"])[0:NB_REAL]
    return out
